# revision 14
# baseline (speedup 1.0000x reference)
"""HGConv kernel for Trainium2: 8-way data-parallel over batch.

Math (per batch b, transposed [d, e] layout so softmaxes reduce the free axis):
    aggT[d,e]  = sum_m nf[m,d] * inc[m,e]            (the ONLY big matmul)
    scoresT    = W_att @ aggT
    t          = exp(scoresT) * aggT                 (un-normalized attn * agg)
    mulT       = t * rinv                            (rinv = 1/rowsum(exp))
    a[e]       = w_eff @ mulT          # w_eff = ec_att_w @ W_proj (host-folded)
    w          = softmax_e(a)
    q[d]       = sum_e mulT[d,e] * w[e]
    logits     = W3 @ q + b2           # W3 = fc_w @ ec_proj_w @ W_proj (host-folded)
  (pooled = sum_e (W_proj@mulT)*w = W_proj @ (mulT @ w) -- so the [d,e]-sized
   edge_feat tensor is never materialized; the e-reduction happens on mulT.)

Engineering notes:
  - inc is 0/1 -> host-cast to fp8_e4m3 (EXACT), quartering the dominant
    HBM stream (16.8 MB -> 4.2 MB/core); nf host-cast to bf16.
  - single bf16(nf) x fp8(inc) matmul per m-chunk half, fp32 PSUM accum;
    no on-device casts in the main loop at all.
  - operands packed on host as per-DMA-group CONTIGUOUS DRAM blocks so the
    SDMA engines read sequentially; streamed across BOTH HWDGE rings
    (sync + scalar) in parallel.  Two 1-chunk head groups let the PE start
    ~1.5us earlier; no SWDGE/gpsimd involvement anywhere.
  - HAM warm-up: a few zero matmuls accumulate 0 into the agg PSUM group
    while the first groups stream, so the PE clock is at 2.4 GHz when real
    data lands (cold PE at 1.2 GHz cannot keep up with the DMA stream).
  - w_eff enters as a [128,128] column-replicated stationary so a[e] is
    computed already broadcast across partitions (no [1,E] row ops).
  - tail elementwise in bf16 (2x/4x DVE modes), tail matmul moving operands
    bf16 (1 cycle/row vs 4 for fp32); exp skips max-subtraction
    (|scores|<=~45, f32-safe; checked on the input distribution).
"""

import sys

import numpy as np

sys.path.insert(0, "/opt/trn_rl_repo")

B, M, E, D, NCAT = 8, 4096, 1024, 128, 64
P = 128
NCHUNK = M // P                      # 32 m-chunks of 128
NHEAD = 2                            # single-chunk head groups
NMAIN = 6                            # 5-chunk main groups
MAINC = 5
assert NHEAD + NMAIN * MAINC == NCHUNK
EH = 512                             # PSUM bank width in fp32

_cache = {}


def _build_nc():
    import concourse.bacc as bacc
    import concourse.bass as bass
    import concourse.mybir as mybir
    from concourse.tile import TileContext

    f32 = mybir.dt.float32
    bf16 = mybir.dt.bfloat16
    fp8 = mybir.dt.float8e4
    AF = mybir.ActivationFunctionType
    ALU = mybir.AluOpType

    nc = bacc.Bacc(None)

    # host-packed operands: each DMA group is one contiguous DRAM block
    inc_h = nc.dram_tensor("inc_h", [NHEAD, P, E], fp8, kind="ExternalInput")
    inc_m = nc.dram_tensor("inc_m", [NMAIN, P, MAINC * E], fp8,
                           kind="ExternalInput")
    nf_h = nc.dram_tensor("nf_h", [NHEAD, P, D], bf16, kind="ExternalInput")
    nf_m = nc.dram_tensor("nf_m", [NMAIN, P, MAINC * D], bf16,
                          kind="ExternalInput")
    # wpack cols: w_attT(128) | w_eff_rep(128) | w3T(64)
    wpack = nc.dram_tensor("wpack", [P, 320], bf16, kind="ExternalInput")
    b2 = nc.dram_tensor("b2_col", [NCAT, 1], f32, kind="ExternalInput")
    out_d = nc.dram_tensor("logits", [NCAT, 1], f32, kind="ExternalOutput")

    with TileContext(nc) as tc:
        with (
            tc.tile_pool(name="const", bufs=1) as cpool,
            tc.tile_pool(name="work", bufs=1) as work,
            tc.tile_pool(name="psb", bufs=2, space=bass.MemorySpace.PSUM) as psb,
            tc.tile_pool(name="pss", bufs=1, space=bass.MemorySpace.PSUM) as pss,
        ):
            inc_sb = cpool.tile([P, NCHUNK, E], fp8)
            nf_sb = cpool.tile([P, NCHUNK, D], bf16)
            wpack_sb = cpool.tile([P, 320], bf16)
            b2_sb = cpool.tile([NCAT, 1], f32)
            warm_sb = cpool.tile([P, EH], bf16)
            nc.vector.memset(warm_sb[:], 0.0)

            # stream across BOTH HWDGE rings; head groups first so the PE
            # can start on chunk 0 while the big main groups stream.
            nf_hr = nf_h.rearrange("g p d -> p g d")
            nf_mr = nf_m.rearrange("g p (c d) -> p g c d", c=MAINC)
            inc_mr = inc_m.rearrange("g p (c e) -> p g c e", c=MAINC)
            nc.scalar.dma_start(nf_sb[:, 0:NHEAD, :], nf_hr[:])
            nc.sync.dma_start(inc_sb[:, 0, :], inc_h.rearrange("g p e -> p g e")[:, 0, :])
            nc.scalar.dma_start(inc_sb[:, 1, :], inc_h.rearrange("g p e -> p g e")[:, 1, :])
            nc.scalar.dma_start(nf_sb[:, 2:17, :], nf_mr[:, 0:3])
            for g in range(NMAIN):
                ring = nc.sync if g % 2 == 0 else nc.scalar
                n0 = NHEAD + g * MAINC
                ring.dma_start(inc_sb[:, n0:n0 + MAINC, :], inc_mr[:, g])
                if g == 1:
                    nc.scalar.dma_start(nf_sb[:, 17:32, :], nf_mr[:, 3:6])
                if g == 3:
                    nc.sync.dma_start(wpack_sb[:], wpack[:])
                    nc.scalar.dma_start(b2_sb[:], b2[:])

            # ---- aggT[d,e] accumulation (warm-up zeros + 32 m-chunks) ----
            agg_ps = psb.tile([P, E], f32, tag="big")
            NWARM = 6
            for i in range(NWARM):
                half = slice(0, EH) if i % 2 == 0 else slice(EH, E)
                nc.tensor.matmul(
                    agg_ps[:, half], warm_sb[:, 0:P], warm_sb[:],
                    start=(i < 2), stop=False,
                )
            for n in range(NCHUNK):
                lhs = nf_sb[:, n, :]
                last = n == NCHUNK - 1
                nc.tensor.matmul(
                    agg_ps[:, 0:EH], lhs, inc_sb[:, n, 0:EH],
                    start=False, stop=last,
                )
                nc.tensor.matmul(
                    agg_ps[:, EH:E], lhs, inc_sb[:, n, EH:E],
                    start=False, stop=last,
                )

            w_attT_sb = wpack_sb[:, 0:128]
            weffr_sb = wpack_sb[:, 128:256]
            w3T_sb = wpack_sb[:, 256:320]

            # ---- tail, pipelined in E-halves where it helps ----
            agg_sb = work.tile([P, E], bf16)
            scr_ps = psb.tile([P, E], f32, tag="big")
            exp_sb = work.tile([P, E], bf16)
            rsum0 = work.tile([P, 1], f32)
            rsum1 = work.tile([P, 1], f32)
            # h1 via ACT, h0 via DVE so both copies overlap; scr/exp chase
            # each half as it lands.
            nc.scalar.copy(agg_sb[:, EH:E], agg_ps[:, EH:E])
            nc.vector.tensor_copy(agg_sb[:, 0:EH], agg_ps[:, 0:EH])
            nc.tensor.matmul(scr_ps[:, EH:E], w_attT_sb, agg_sb[:, EH:E],
                             start=True, stop=True)
            nc.tensor.matmul(scr_ps[:, 0:EH], w_attT_sb, agg_sb[:, 0:EH],
                             start=True, stop=True)
            nc.scalar.activation(exp_sb[:, EH:E], scr_ps[:, EH:E], AF.Exp,
                                 bias=0.0, accum_out=rsum1[:])
            nc.scalar.activation(exp_sb[:, 0:EH], scr_ps[:, 0:EH], AF.Exp,
                                 bias=0.0, accum_out=rsum0[:])
            t_sb = work.tile([P, E], bf16)
            nc.vector.tensor_mul(t_sb[:, EH:E], exp_sb[:, EH:E],
                                 agg_sb[:, EH:E])
            nc.vector.tensor_mul(t_sb[:, 0:EH], exp_sb[:, 0:EH],
                                 agg_sb[:, 0:EH])
            rsum = work.tile([P, 1], f32)
            nc.vector.tensor_add(rsum[:], rsum0[:], rsum1[:])
            rinv = work.tile([P, 1], f32)
            nc.vector.reciprocal(rinv[:], rsum[:])
            mul_sb = work.tile([P, E], bf16)
            nc.vector.tensor_scalar_mul(mul_sb[:], t_sb[:], rinv[:])

            # ---- a (row-replicated) = w_eff @ mulT ; softmax over e ----
            ab_ps = psb.tile([P, E], f32, tag="big")
            nc.tensor.matmul(ab_ps[:, 0:EH], weffr_sb, mul_sb[:, 0:EH],
                             start=True, stop=True)
            nc.tensor.matmul(ab_ps[:, EH:E], weffr_sb, mul_sb[:, EH:E],
                             start=True, stop=True)
            expb = work.tile([P, E], bf16)
            asum = work.tile([P, 1], f32)
            nc.scalar.activation(expb[:], ab_ps[:], AF.Exp,
                                 bias=0.0, accum_out=asum[:])
            ainv = work.tile([P, 1], f32)
            nc.vector.reciprocal(ainv[:], asum[:])

            # ---- q = mulT @ w ; logits = W3 @ q + b2 ----
            prod = work.tile([P, E], bf16)
            nc.vector.tensor_mul(prod[:], mul_sb[:], expb[:])
            sink = work.tile([P, E], bf16)
            q_raw = work.tile([P, 1], f32)
            nc.vector.tensor_scalar(
                sink[:], prod[:], 1.0, 0.0, op0=ALU.mult, op1=ALU.add,
                accum_out=q_raw[:],
            )
            q_sb = work.tile([P, 1], bf16)
            nc.vector.tensor_scalar_mul(q_sb[:], q_raw[:], ainv[:])
            log_ps = pss.tile([NCAT, 1], f32, tag="tiny")
            nc.tensor.matmul(log_ps[:], w3T_sb, q_sb[:], start=True, stop=True)
            logit_sb = work.tile([NCAT, 1], f32)
            nc.vector.tensor_add(logit_sb[:], log_ps[:], b2_sb[:])
            nc.sync.dma_start(out_d[:], logit_sb[:])

    nc.finalize()
    return nc


def _get_nc():
    if "nc" not in _cache:
        _cache["nc"] = _build_nc()
    return _cache["nc"]


def kernel(node_feats, inc_mat, W_att, W_proj, ec_att_w, ec_proj_w, ec_proj_b,
           fc_w, fc_b, **trace_kw):
    import ml_dtypes
    from concourse.bass_utils import run_bass_kernel_spmd

    node_feats = np.asarray(node_feats, dtype=np.float32)
    inc_mat = np.asarray(inc_mat, dtype=np.float32)
    W_att = np.asarray(W_att, np.float32)
    W_proj = np.asarray(W_proj, np.float32)
    ec_att_w = np.asarray(ec_att_w, np.float32)
    ec_proj_w = np.asarray(ec_proj_w, np.float32)
    ec_proj_b = np.asarray(ec_proj_b, np.float32)
    fc_w = np.asarray(fc_w, np.float32)
    fc_b = np.asarray(fc_b, np.float32)

    # host-folded weights (constant preprocessing, O(D^2) flops)
    w_eff = (ec_att_w @ W_proj).ravel()                       # [D]
    W3 = fc_w @ ec_proj_w @ W_proj                            # [NCAT, D]
    b2 = (fc_w @ ec_proj_b + fc_b).reshape(NCAT, 1)           # [NCAT, 1]
    wpack = np.concatenate(
        [
            np.ascontiguousarray(W_att.T),                    # [D, D]
            np.tile(w_eff[:, None], (1, D)),                  # [D, D] replicated
            np.ascontiguousarray(W3.T),                       # [D, NCAT]
        ],
        axis=1,
    ).astype(ml_dtypes.bfloat16)

    # pack per-core operands into contiguous per-DMA-group blocks
    nf4 = node_feats.reshape(B, NCHUNK, P, D)
    inc4 = inc_mat.reshape(B, NCHUNK, P, E)
    nf_h = nf4[:, :NHEAD].astype(ml_dtypes.bfloat16)          # [B,2,P,D]
    inc_h = inc4[:, :NHEAD].astype(ml_dtypes.float8_e4m3)     # [B,2,P,E]
    nf_m = (nf4[:, NHEAD:].reshape(B, NMAIN, MAINC, P, D)
            .transpose(0, 1, 3, 2, 4).reshape(B, NMAIN, P, MAINC * D)
            .astype(ml_dtypes.bfloat16))
    inc_m = (inc4[:, NHEAD:].reshape(B, NMAIN, MAINC, P, E)
             .transpose(0, 1, 3, 2, 4).reshape(B, NMAIN, P, MAINC * E)
             .astype(ml_dtypes.float8_e4m3))

    shared = {"wpack": wpack, "b2_col": np.ascontiguousarray(b2)}
    in_maps = [
        {"nf_h": np.ascontiguousarray(nf_h[b]),
         "nf_m": np.ascontiguousarray(nf_m[b]),
         "inc_h": np.ascontiguousarray(inc_h[b]),
         "inc_m": np.ascontiguousarray(inc_m[b]), **shared}
        for b in range(B)
    ]
    res = run_bass_kernel_spmd(_get_nc(), in_maps, list(range(B)), **trace_kw)
    out = np.stack([res.results[b]["logits"].reshape(NCAT) for b in range(B)])
    if trace_kw:
        return out, res
    return out


# revision 16
# speedup vs baseline: 1.0485x; 1.0485x over previous
"""HGConv kernel for Trainium2: 8-way data-parallel over batch.

Math (per batch b, transposed [d, e] layout so softmaxes reduce the free axis):
    aggT[d,e]  = sum_m nf[m,d] * inc[m,e]            (the ONLY big matmul)
    scoresT    = W_att @ aggT
    t          = exp(scoresT) * aggT                 (un-normalized attn * agg)
    mulT       = t * rinv                            (rinv = 1/rowsum(exp))
    a[e]       = w_eff @ mulT          # w_eff = ec_att_w @ W_proj (host-folded)
    w          = softmax_e(a)
    q[d]       = sum_e mulT[d,e] * w[e]
    logits     = W3 @ q + b2           # W3 = fc_w @ ec_proj_w @ W_proj (host-folded)
  (pooled = sum_e (W_proj@mulT)*w = W_proj @ (mulT @ w) -- so the [d,e]-sized
   edge_feat tensor is never materialized; the e-reduction happens on mulT.)

Engineering notes:
  - inc is 0/1 -> host-cast to fp8_e4m3 (EXACT), quartering the dominant
    HBM stream (16.8 MB -> 4.2 MB/core); nf host-cast to bf16.
  - single bf16(nf) x fp8(inc) matmul per m-chunk half, fp32 PSUM accum;
    no on-device casts in the main loop at all.
  - operands packed on host as per-DMA-group CONTIGUOUS DRAM blocks so the
    SDMA engines read sequentially; streamed across BOTH HWDGE rings
    (sync + scalar) in parallel.  Two 1-chunk head groups let the PE start
    ~1.5us earlier; no SWDGE/gpsimd involvement anywhere.
  - HAM warm-up: a few zero matmuls accumulate 0 into the agg PSUM group
    while the first groups stream, so the PE clock is at 2.4 GHz when real
    data lands (cold PE at 1.2 GHz cannot keep up with the DMA stream).
  - w_eff enters as a [128,128] column-replicated stationary so a[e] is
    computed already broadcast across partitions (no [1,E] row ops).
  - tail elementwise in bf16 (2x/4x DVE modes), tail matmul moving operands
    bf16 (1 cycle/row vs 4 for fp32); exp skips max-subtraction
    (|scores|<=~45, f32-safe; checked on the input distribution).
"""

import sys

import numpy as np

sys.path.insert(0, "/opt/trn_rl_repo")

B, M, E, D, NCAT = 8, 4096, 1024, 128, 64
P = 128
NCHUNK = M // P                      # 32 m-chunks of 128
NHEAD = 2                            # single-chunk head groups
NMAIN = 6                            # 5-chunk main groups
MAINC = 5
assert NHEAD + NMAIN * MAINC == NCHUNK
EH = 512                             # PSUM bank width in fp32

_cache = {}


def _build_nc():
    import concourse.bacc as bacc
    import concourse.bass as bass
    import concourse.mybir as mybir
    from concourse.tile import TileContext

    f32 = mybir.dt.float32
    bf16 = mybir.dt.bfloat16
    fp8 = mybir.dt.float8e4
    AF = mybir.ActivationFunctionType
    ALU = mybir.AluOpType

    nc = bacc.Bacc(None)

    # host-packed operands: each DMA group is one contiguous DRAM block
    inc_h = nc.dram_tensor("inc_h", [NHEAD, P, E], fp8, kind="ExternalInput")
    inc_m = nc.dram_tensor("inc_m", [NMAIN, P, MAINC * E], fp8,
                           kind="ExternalInput")
    nf_h = nc.dram_tensor("nf_h", [NHEAD, P, D], bf16, kind="ExternalInput")
    nf_m = nc.dram_tensor("nf_m", [NMAIN, P, MAINC * D], bf16,
                          kind="ExternalInput")
    # wpack cols: w_attT(128) | w_eff_rep(128) | w3T(64)
    wpack = nc.dram_tensor("wpack", [P, 320], bf16, kind="ExternalInput")
    b2 = nc.dram_tensor("b2_col", [NCAT, 1], f32, kind="ExternalInput")
    out_d = nc.dram_tensor("logits", [NCAT, 1], f32, kind="ExternalOutput")

    with TileContext(nc) as tc:
        with (
            tc.tile_pool(name="const", bufs=1) as cpool,
            tc.tile_pool(name="work", bufs=1) as work,
            tc.tile_pool(name="psb", bufs=2, space=bass.MemorySpace.PSUM) as psb,
            tc.tile_pool(name="pss", bufs=1, space=bass.MemorySpace.PSUM) as pss,
        ):
            inc_sb = cpool.tile([P, NCHUNK, E], fp8)
            nf_sb = cpool.tile([P, NCHUNK, D], bf16)
            wpack_sb = cpool.tile([P, 320], bf16)
            b2_sb = cpool.tile([NCAT, 1], f32)
            warm_sb = cpool.tile([P, EH], bf16)
            nc.vector.memset(warm_sb[:], 0.0)

            # inc streams on the sync ring ONLY, in strict consumption order
            # (a queue's DMAs drain FIFO, so arrival order == matmul order);
            # nf + weights ride the scalar HWDGE ring and finish early.
            nf_hr = nf_h.rearrange("g p d -> p g d")
            nf_mr = nf_m.rearrange("g p (c d) -> p g c d", c=MAINC)
            inc_hr = inc_h.rearrange("g p e -> p g e")
            inc_mr = inc_m.rearrange("g p (c e) -> p g c e", c=MAINC)
            nc.scalar.dma_start(nf_sb[:, 0:NHEAD, :], nf_hr[:])
            nc.sync.dma_start(inc_sb[:, 0, :], inc_hr[:, 0, :])
            nc.sync.dma_start(inc_sb[:, 1, :], inc_hr[:, 1, :])
            nc.scalar.dma_start(nf_sb[:, 2:17, :], nf_mr[:, 0:3])
            nc.scalar.dma_start(nf_sb[:, 17:32, :], nf_mr[:, 3:6])
            nc.scalar.dma_start(wpack_sb[:], wpack[:])
            nc.scalar.dma_start(b2_sb[:], b2[:])
            for g in range(NMAIN):
                n0 = NHEAD + g * MAINC
                nc.sync.dma_start(inc_sb[:, n0:n0 + MAINC, :], inc_mr[:, g])

            # ---- aggT[d,e] accumulation (warm-up zeros + 32 m-chunks) ----
            agg_ps = psb.tile([P, E], f32, tag="big")
            NWARM = 6
            for i in range(NWARM):
                half = slice(0, EH) if i % 2 == 0 else slice(EH, E)
                nc.tensor.matmul(
                    agg_ps[:, half], warm_sb[:, 0:P], warm_sb[:],
                    start=(i < 2), stop=False,
                )
            for n in range(NCHUNK):
                lhs = nf_sb[:, n, :]
                last = n == NCHUNK - 1
                nc.tensor.matmul(
                    agg_ps[:, 0:EH], lhs, inc_sb[:, n, 0:EH],
                    start=False, stop=last,
                )
                nc.tensor.matmul(
                    agg_ps[:, EH:E], lhs, inc_sb[:, n, EH:E],
                    start=False, stop=last,
                )

            w_attT_sb = wpack_sb[:, 0:128]
            weffr_sb = wpack_sb[:, 128:256]
            w3T_sb = wpack_sb[:, 256:320]

            # ---- tail ----
            # t = exp(scores)*agg stays un-normalized; the per-row 1/rsum
            # folds into the ab stationary (w2 = w_eff*rinv) and the q
            # scale; the UNIFORM 1/asum folds into the final bias-add STT.
            agg_sb = work.tile([P, E], bf16)
            scr_ps = psb.tile([P, E], f32, tag="big")
            exp_sb = work.tile([P, E], bf16)
            rsum = work.tile([P, 1], f32)
            nc.scalar.copy(agg_sb[:, EH:E], agg_ps[:, EH:E])
            nc.vector.tensor_copy(agg_sb[:, 0:EH], agg_ps[:, 0:EH])
            nc.tensor.matmul(scr_ps[:, EH:E], w_attT_sb, agg_sb[:, EH:E],
                             start=True, stop=True)
            nc.tensor.matmul(scr_ps[:, 0:EH], w_attT_sb, agg_sb[:, 0:EH],
                             start=True, stop=True)
            nc.scalar.activation(exp_sb[:], scr_ps[:], AF.Exp,
                                 bias=0.0, accum_out=rsum[:])
            rinv = work.tile([P, 1], f32)
            nc.vector.reciprocal(rinv[:], rsum[:])
            t_sb = work.tile([P, E], bf16)
            nc.vector.tensor_mul(t_sb[:], exp_sb[:], agg_sb[:])
            w2_sb = work.tile([P, P], bf16)
            nc.vector.tensor_scalar_mul(w2_sb[:], weffr_sb, rinv[:])

            # ---- a (row-replicated) = (w_eff*rinv) @ t ; softmax over e ----
            ab_ps = psb.tile([P, E], f32, tag="big")
            nc.tensor.matmul(ab_ps[:, 0:EH], w2_sb[:], t_sb[:, 0:EH],
                             start=True, stop=True)
            nc.tensor.matmul(ab_ps[:, EH:E], w2_sb[:], t_sb[:, EH:E],
                             start=True, stop=True)
            expb = work.tile([P, E], bf16)
            asum = work.tile([P, 1], f32)
            nc.scalar.activation(expb[:], ab_ps[:], AF.Exp,
                                 bias=0.0, accum_out=asum[:])
            ainv = work.tile([P, 1], f32)
            nc.vector.reciprocal(ainv[:], asum[:])

            # ---- q = (t @ w) * rinv ; logits = (W3 @ q) * ainv + b2 ----
            prod = work.tile([P, E], bf16)
            nc.vector.tensor_mul(prod[:], t_sb[:], expb[:])
            sink = work.tile([P, E], bf16)
            q_raw = work.tile([P, 1], f32)
            nc.vector.tensor_scalar(
                sink[:], prod[:], 1.0, 0.0, op0=ALU.mult, op1=ALU.add,
                accum_out=q_raw[:],
            )
            q_sb = work.tile([P, 1], bf16)
            nc.vector.tensor_scalar_mul(q_sb[:], q_raw[:], rinv[:])
            log_ps = pss.tile([NCAT, 1], f32, tag="tiny")
            nc.tensor.matmul(log_ps[:], w3T_sb, q_sb[:], start=True, stop=True)
            logit_sb = work.tile([NCAT, 1], f32)
            nc.vector.scalar_tensor_tensor(
                logit_sb[:], log_ps[:], ainv[0:NCAT, :], b2_sb[:],
                op0=ALU.mult, op1=ALU.add,
            )
            nc.sync.dma_start(out_d[:], logit_sb[:])

    nc.finalize()
    return nc


def _get_nc():
    if "nc" not in _cache:
        _cache["nc"] = _build_nc()
    return _cache["nc"]


def kernel(node_feats, inc_mat, W_att, W_proj, ec_att_w, ec_proj_w, ec_proj_b,
           fc_w, fc_b, **trace_kw):
    import ml_dtypes
    from concourse.bass_utils import run_bass_kernel_spmd

    node_feats = np.asarray(node_feats, dtype=np.float32)
    inc_mat = np.asarray(inc_mat, dtype=np.float32)
    W_att = np.asarray(W_att, np.float32)
    W_proj = np.asarray(W_proj, np.float32)
    ec_att_w = np.asarray(ec_att_w, np.float32)
    ec_proj_w = np.asarray(ec_proj_w, np.float32)
    ec_proj_b = np.asarray(ec_proj_b, np.float32)
    fc_w = np.asarray(fc_w, np.float32)
    fc_b = np.asarray(fc_b, np.float32)

    # host-folded weights (constant preprocessing, O(D^2) flops)
    w_eff = (ec_att_w @ W_proj).ravel()                       # [D]
    W3 = fc_w @ ec_proj_w @ W_proj                            # [NCAT, D]
    b2 = (fc_w @ ec_proj_b + fc_b).reshape(NCAT, 1)           # [NCAT, 1]
    wpack = np.concatenate(
        [
            np.ascontiguousarray(W_att.T),                    # [D, D]
            np.tile(w_eff[:, None], (1, D)),                  # [D, D] replicated
            np.ascontiguousarray(W3.T),                       # [D, NCAT]
        ],
        axis=1,
    ).astype(ml_dtypes.bfloat16)

    # pack per-core operands into contiguous per-DMA-group blocks
    nf4 = node_feats.reshape(B, NCHUNK, P, D)
    inc4 = inc_mat.reshape(B, NCHUNK, P, E)
    nf_h = nf4[:, :NHEAD].astype(ml_dtypes.bfloat16)          # [B,2,P,D]
    inc_h = inc4[:, :NHEAD].astype(ml_dtypes.float8_e4m3)     # [B,2,P,E]
    nf_m = (nf4[:, NHEAD:].reshape(B, NMAIN, MAINC, P, D)
            .transpose(0, 1, 3, 2, 4).reshape(B, NMAIN, P, MAINC * D)
            .astype(ml_dtypes.bfloat16))
    inc_m = (inc4[:, NHEAD:].reshape(B, NMAIN, MAINC, P, E)
             .transpose(0, 1, 3, 2, 4).reshape(B, NMAIN, P, MAINC * E)
             .astype(ml_dtypes.float8_e4m3))

    shared = {"wpack": wpack, "b2_col": np.ascontiguousarray(b2)}
    in_maps = [
        {"nf_h": np.ascontiguousarray(nf_h[b]),
         "nf_m": np.ascontiguousarray(nf_m[b]),
         "inc_h": np.ascontiguousarray(inc_h[b]),
         "inc_m": np.ascontiguousarray(inc_m[b]), **shared}
        for b in range(B)
    ]
    res = run_bass_kernel_spmd(_get_nc(), in_maps, list(range(B)), **trace_kw)
    out = np.stack([res.results[b]["logits"].reshape(NCAT) for b in range(B)])
    if trace_kw:
        return out, res
    return out


# revision 17
# speedup vs baseline: 1.0658x; 1.0166x over previous
"""HGConv kernel for Trainium2: 8-way data-parallel over batch.

Math (per batch b, transposed [d, e] layout so softmaxes reduce the free axis):
    aggT[d,e]  = sum_m nf[m,d] * inc[m,e]            (the ONLY big matmul)
    scoresT    = W_att @ aggT
    t          = exp(scoresT) * aggT                 (un-normalized attn * agg)
    mulT       = t * rinv                            (rinv = 1/rowsum(exp))
    a[e]       = w_eff @ mulT          # w_eff = ec_att_w @ W_proj (host-folded)
    w          = softmax_e(a)
    q[d]       = sum_e mulT[d,e] * w[e]
    logits     = W3 @ q + b2           # W3 = fc_w @ ec_proj_w @ W_proj (host-folded)
  (pooled = sum_e (W_proj@mulT)*w = W_proj @ (mulT @ w) -- so the [d,e]-sized
   edge_feat tensor is never materialized; the e-reduction happens on mulT.)

Engineering notes:
  - inc is 0/1 -> host-cast to fp8_e4m3 (EXACT), quartering the dominant
    HBM stream (16.8 MB -> 4.2 MB/core); nf host-cast to bf16.
  - single bf16(nf) x fp8(inc) matmul per m-chunk half, fp32 PSUM accum;
    no on-device casts in the main loop at all.
  - operands packed on host as per-DMA-group CONTIGUOUS DRAM blocks so the
    SDMA engines read sequentially; streamed across BOTH HWDGE rings
    (sync + scalar) in parallel.  Two 1-chunk head groups let the PE start
    ~1.5us earlier; no SWDGE/gpsimd involvement anywhere.
  - HAM warm-up: a few zero matmuls accumulate 0 into the agg PSUM group
    while the first groups stream, so the PE clock is at 2.4 GHz when real
    data lands (cold PE at 1.2 GHz cannot keep up with the DMA stream).
  - w_eff enters as a [128,128] column-replicated stationary so a[e] is
    computed already broadcast across partitions (no [1,E] row ops).
  - tail elementwise in bf16 (2x/4x DVE modes), tail matmul moving operands
    bf16 (1 cycle/row vs 4 for fp32); exp skips max-subtraction
    (|scores|<=~45, f32-safe; checked on the input distribution).
"""

import sys

import numpy as np

sys.path.insert(0, "/opt/trn_rl_repo")

B, M, E, D, NCAT = 8, 4096, 1024, 128, 64
P = 128
NCHUNK = M // P                      # 32 m-chunks of 128
NHEAD = 2                            # single-chunk head groups
NMAIN = 6                            # 5-chunk main groups
MAINC = 5
assert NHEAD + NMAIN * MAINC == NCHUNK
EH = 512                             # PSUM bank width in fp32

_cache = {}


def _build_nc():
    import concourse.bacc as bacc
    import concourse.bass as bass
    import concourse.mybir as mybir
    from concourse.tile import TileContext

    f32 = mybir.dt.float32
    bf16 = mybir.dt.bfloat16
    fp8 = mybir.dt.float8e4
    AF = mybir.ActivationFunctionType
    ALU = mybir.AluOpType

    nc = bacc.Bacc(None)

    # host-packed operands: each DMA group is one contiguous DRAM block
    inc_h = nc.dram_tensor("inc_h", [NHEAD, P, E], fp8, kind="ExternalInput")
    inc_m = nc.dram_tensor("inc_m", [NMAIN, P, MAINC * E], fp8,
                           kind="ExternalInput")
    nf_h = nc.dram_tensor("nf_h", [NHEAD, P, D], bf16, kind="ExternalInput")
    nf_m = nc.dram_tensor("nf_m", [NMAIN, P, MAINC * D], bf16,
                          kind="ExternalInput")
    # wpack cols: w_attT(128) | w_eff_rep(128) | w3T(64)
    wpack = nc.dram_tensor("wpack", [P, 320], bf16, kind="ExternalInput")
    b2 = nc.dram_tensor("b2_col", [NCAT, 1], f32, kind="ExternalInput")
    out_d = nc.dram_tensor("logits", [NCAT, 1], f32, kind="ExternalOutput")

    with TileContext(nc) as tc:
        with (
            tc.tile_pool(name="const", bufs=1) as cpool,
            tc.tile_pool(name="work", bufs=1) as work,
            tc.tile_pool(name="psb", bufs=2, space=bass.MemorySpace.PSUM) as psb,
            tc.tile_pool(name="pss", bufs=1, space=bass.MemorySpace.PSUM) as pss,
        ):
            inc_sb = cpool.tile([P, NCHUNK, E], fp8)
            nf_sb = cpool.tile([P, NCHUNK, D], bf16)
            wpack_sb = cpool.tile([P, 320], bf16)
            b2_sb = cpool.tile([NCAT, 1], f32)
            warm_sb = cpool.tile([P, EH], bf16)
            nc.vector.memset(warm_sb[:], 0.0)

            # ALL bulk data rides the single sync queue in strict consumption
            # order (a queue's DMAs drain FIFO at full bandwidth, so arrival
            # order == matmul order); only the tiny weights use the scalar
            # ring.
            nf_hr = nf_h.rearrange("g p d -> p g d")
            nf_mr = nf_m.rearrange("g p (c d) -> p g c d", c=MAINC)
            inc_hr = inc_h.rearrange("g p e -> p g e")
            inc_mr = inc_m.rearrange("g p (c e) -> p g c e", c=MAINC)
            nc.sync.dma_start(nf_sb[:, 0:NHEAD, :], nf_hr[:])
            nc.sync.dma_start(inc_sb[:, 0, :], inc_hr[:, 0, :])
            nc.sync.dma_start(inc_sb[:, 1, :], inc_hr[:, 1, :])
            nc.scalar.dma_start(wpack_sb[:], wpack[:])
            nc.scalar.dma_start(b2_sb[:], b2[:])
            for g in range(NMAIN):
                if g == 0:
                    nc.sync.dma_start(nf_sb[:, 2:17, :], nf_mr[:, 0:3])
                if g == 3:
                    nc.sync.dma_start(nf_sb[:, 17:32, :], nf_mr[:, 3:6])
                n0 = NHEAD + g * MAINC
                nc.sync.dma_start(inc_sb[:, n0:n0 + MAINC, :], inc_mr[:, g])

            # ---- aggT[d,e] accumulation (warm-up zeros + 32 m-chunks) ----
            agg_ps = psb.tile([P, E], f32, tag="big")
            NWARM = 6
            for i in range(NWARM):
                half = slice(0, EH) if i % 2 == 0 else slice(EH, E)
                nc.tensor.matmul(
                    agg_ps[:, half], warm_sb[:, 0:P], warm_sb[:],
                    start=(i < 2), stop=False,
                )
            for n in range(NCHUNK):
                lhs = nf_sb[:, n, :]
                last = n == NCHUNK - 1
                nc.tensor.matmul(
                    agg_ps[:, 0:EH], lhs, inc_sb[:, n, 0:EH],
                    start=False, stop=last,
                )
                nc.tensor.matmul(
                    agg_ps[:, EH:E], lhs, inc_sb[:, n, EH:E],
                    start=False, stop=last,
                )

            w_attT_sb = wpack_sb[:, 0:128]
            weffr_sb = wpack_sb[:, 128:256]
            w3T_sb = wpack_sb[:, 256:320]

            # ---- tail ----
            # t = exp(scores)*agg stays un-normalized; the per-row 1/rsum
            # folds into the ab stationary (w2 = w_eff*rinv) and the q
            # scale; the UNIFORM 1/asum folds into the final bias-add STT.
            agg_sb = work.tile([P, E], bf16)
            scr_ps = psb.tile([P, E], f32, tag="big")
            exp_sb = work.tile([P, E], bf16)
            rsum = work.tile([P, 1], f32)
            nc.scalar.copy(agg_sb[:, EH:E], agg_ps[:, EH:E])
            nc.vector.tensor_copy(agg_sb[:, 0:EH], agg_ps[:, 0:EH])
            nc.tensor.matmul(scr_ps[:, EH:E], w_attT_sb, agg_sb[:, EH:E],
                             start=True, stop=True)
            nc.tensor.matmul(scr_ps[:, 0:EH], w_attT_sb, agg_sb[:, 0:EH],
                             start=True, stop=True)
            nc.scalar.activation(exp_sb[:], scr_ps[:], AF.Exp,
                                 bias=0.0, accum_out=rsum[:])
            rinv = work.tile([P, 1], f32)
            nc.vector.reciprocal(rinv[:], rsum[:])
            t_sb = work.tile([P, E], bf16)
            nc.vector.tensor_mul(t_sb[:], exp_sb[:], agg_sb[:])
            w2_sb = work.tile([P, P], bf16)
            nc.vector.tensor_scalar_mul(w2_sb[:], weffr_sb, rinv[:])

            # ---- a (row-replicated) = (w_eff*rinv) @ t ; softmax over e ----
            ab_ps = psb.tile([P, E], f32, tag="big")
            nc.tensor.matmul(ab_ps[:, 0:EH], w2_sb[:], t_sb[:, 0:EH],
                             start=True, stop=True)
            nc.tensor.matmul(ab_ps[:, EH:E], w2_sb[:], t_sb[:, EH:E],
                             start=True, stop=True)
            expb = work.tile([P, E], bf16)
            asum = work.tile([P, 1], f32)
            nc.scalar.activation(expb[:], ab_ps[:], AF.Exp,
                                 bias=0.0, accum_out=asum[:])
            ainv = work.tile([P, 1], f32)
            nc.vector.reciprocal(ainv[:], asum[:])

            # ---- q = (t @ w) * rinv ; logits = (W3 @ q) * ainv + b2 ----
            prod = work.tile([P, E], bf16)
            nc.vector.tensor_mul(prod[:], t_sb[:], expb[:])
            sink = work.tile([P, E], bf16)
            q_raw = work.tile([P, 1], f32)
            nc.vector.tensor_scalar(
                sink[:], prod[:], 1.0, 0.0, op0=ALU.mult, op1=ALU.add,
                accum_out=q_raw[:],
            )
            q_sb = work.tile([P, 1], bf16)
            nc.vector.tensor_scalar_mul(q_sb[:], q_raw[:], rinv[:])
            log_ps = pss.tile([NCAT, 1], f32, tag="tiny")
            nc.tensor.matmul(log_ps[:], w3T_sb, q_sb[:], start=True, stop=True)
            logit_sb = work.tile([NCAT, 1], f32)
            nc.vector.scalar_tensor_tensor(
                logit_sb[:], log_ps[:], ainv[0:NCAT, :], b2_sb[:],
                op0=ALU.mult, op1=ALU.add,
            )
            nc.sync.dma_start(out_d[:], logit_sb[:])

    nc.finalize()
    return nc


def _get_nc():
    if "nc" not in _cache:
        _cache["nc"] = _build_nc()
    return _cache["nc"]


def kernel(node_feats, inc_mat, W_att, W_proj, ec_att_w, ec_proj_w, ec_proj_b,
           fc_w, fc_b, **trace_kw):
    import ml_dtypes
    from concourse.bass_utils import run_bass_kernel_spmd

    node_feats = np.asarray(node_feats, dtype=np.float32)
    inc_mat = np.asarray(inc_mat, dtype=np.float32)
    W_att = np.asarray(W_att, np.float32)
    W_proj = np.asarray(W_proj, np.float32)
    ec_att_w = np.asarray(ec_att_w, np.float32)
    ec_proj_w = np.asarray(ec_proj_w, np.float32)
    ec_proj_b = np.asarray(ec_proj_b, np.float32)
    fc_w = np.asarray(fc_w, np.float32)
    fc_b = np.asarray(fc_b, np.float32)

    # host-folded weights (constant preprocessing, O(D^2) flops)
    w_eff = (ec_att_w @ W_proj).ravel()                       # [D]
    W3 = fc_w @ ec_proj_w @ W_proj                            # [NCAT, D]
    b2 = (fc_w @ ec_proj_b + fc_b).reshape(NCAT, 1)           # [NCAT, 1]
    wpack = np.concatenate(
        [
            np.ascontiguousarray(W_att.T),                    # [D, D]
            np.tile(w_eff[:, None], (1, D)),                  # [D, D] replicated
            np.ascontiguousarray(W3.T),                       # [D, NCAT]
        ],
        axis=1,
    ).astype(ml_dtypes.bfloat16)

    # pack per-core operands into contiguous per-DMA-group blocks
    nf4 = node_feats.reshape(B, NCHUNK, P, D)
    inc4 = inc_mat.reshape(B, NCHUNK, P, E)
    nf_h = nf4[:, :NHEAD].astype(ml_dtypes.bfloat16)          # [B,2,P,D]
    inc_h = inc4[:, :NHEAD].astype(ml_dtypes.float8_e4m3)     # [B,2,P,E]
    nf_m = (nf4[:, NHEAD:].reshape(B, NMAIN, MAINC, P, D)
            .transpose(0, 1, 3, 2, 4).reshape(B, NMAIN, P, MAINC * D)
            .astype(ml_dtypes.bfloat16))
    inc_m = (inc4[:, NHEAD:].reshape(B, NMAIN, MAINC, P, E)
             .transpose(0, 1, 3, 2, 4).reshape(B, NMAIN, P, MAINC * E)
             .astype(ml_dtypes.float8_e4m3))

    shared = {"wpack": wpack, "b2_col": np.ascontiguousarray(b2)}
    in_maps = [
        {"nf_h": np.ascontiguousarray(nf_h[b]),
         "nf_m": np.ascontiguousarray(nf_m[b]),
         "inc_h": np.ascontiguousarray(inc_h[b]),
         "inc_m": np.ascontiguousarray(inc_m[b]), **shared}
        for b in range(B)
    ]
    res = run_bass_kernel_spmd(_get_nc(), in_maps, list(range(B)), **trace_kw)
    out = np.stack([res.results[b]["logits"].reshape(NCAT) for b in range(B)])
    if trace_kw:
        return out, res
    return out
